# revision 2
# baseline (speedup 1.0000x reference)
"""Group-equivariant depthwise conv (C4) on 8 Trainium2 NeuronCores — bf16.

out[b, r*C+c] = crosscorr(x[b, c], rot90(weight[c, 0], r)), r in 0..3
x: [16, 192, 128, 128] f32, weight: [192, 1, 3, 3] f32 -> out: [16, 768, 128, 128].

Data-parallel over batch (2 images/core). Host pads+casts x to bf16
[384, 130, 130] (1px zero halo) so chunk loads are single contiguous DMAs;
out is stored bf16 and cast back to f32 on host (rel-err budget 2e-2).
Per rotation, the 9 taps are split between the TensorEngine (diagonal
bf16 stationaries, PSUM accumulation, ACT drains) and the Vector engine
(scalar_tensor_tensor MACs); rotations with no PE taps start from an ACT
scale-copy of the center tap.
"""

import numpy as np
from contextlib import ExitStack

from concourse import bacc, mybir, tile
from concourse.bass_utils import run_bass_kernel_spmd

B, C, H, W = 16, 192, 128, 128
NCORES = 8
BS = B // NCORES
ROWS = BS * C               # 384
NCHUNK = ROWS // 128        # 3
HT = 32
NHT = H // HT               # 4
SUB = 4
NSUB = HT // SUB            # 8
TW = W + 2                  # 130
TH = H + 2                  # 130

F32 = mybir.dt.float32
BF16 = mybir.dt.bfloat16

TAPS = [(1, 1)] + [(ti, tj) for ti in range(3) for tj in range(3) if (ti, tj) != (1, 1)]

CHUNK_SEGS = []
for _ch in range(NCHUNK):
    segs = []
    g = _ch * 128
    while g < (_ch + 1) * 128:
        b_loc, c0 = g // C, g % C
        n = min((_ch + 1) * 128 - g, C - c0)
        segs.append((g - _ch * 128, n, b_loc, c0))
        g += n
    CHUNK_SEGS.append(segs)

# per-rotation (k_pe, m_dve): k+m == 9, or k==0 with m == 8 plus ACT init
SPLIT = {0: (9, 0), 1: (9, 0), 2: (8, 1), 3: (0, 8)}


def _build(split=None, do_stores=True, do_loads=True, load_split=4, alt_dge=True):
    split = split or SPLIT
    nc = bacc.Bacc("TRN2", target_bir_lowering=False, debug=False, num_devices=NCORES)
    x_d = nc.dram_tensor("x", [ROWS, TH, TW], BF16, kind="ExternalInput").ap()
    w_d = nc.dram_tensor("w36", [ROWS, 36], F32, kind="ExternalInput").ap()
    wb_d = nc.dram_tensor("w36b", [ROWS, 36], BF16, kind="ExternalInput").ap()
    o_d = nc.dram_tensor("out", [BS * 4 * C, H, W], BF16, kind="ExternalOutput").ap()

    rots = sorted(split.keys())
    any_pe = any(split[r][0] > 0 for r in rots)

    with tile.TileContext(nc) as tc, ExitStack() as ctx:
        xpool = ctx.enter_context(tc.tile_pool(name="xt", bufs=2))
        opool = ctx.enter_context(tc.tile_pool(name="osb", bufs=4))
        wpool = ctx.enter_context(tc.tile_pool(name="wsb", bufs=2))
        dpool = ctx.enter_context(tc.tile_pool(name="diag", bufs=2))
        pspool = ctx.enter_context(tc.tile_pool(name="ps", bufs=8, space="PSUM"))

        for ch in range(NCHUNK):
            g0 = ch * 128
            w_sb = wpool.tile([128, 36], F32, tag="wsb")
            nc.sync.dma_start(w_sb[:], w_d[g0 : g0 + 128, :])
            if any_pe:
                wb_sb = wpool.tile([128, 36], BF16, tag="wbsb")
                nc.sync.dma_start(wb_sb[:], wb_d[g0 : g0 + 128, :])
                diag = dpool.tile([128, 36, 128], BF16, tag="diag")
                nc.gpsimd.affine_select(
                    out=diag[:],
                    in_=wb_sb[:].broadcast_to([128, 36, 128]),
                    compare_op=mybir.AluOpType.is_equal,
                    fill=0.0,
                    base=0,
                    pattern=[[0, 36], [-1, 128]],
                    channel_multiplier=1,
                )

            xt = xpool.tile([128, TH, TW], BF16, tag="xt")
            if do_loads:
                dges = [nc.sync, nc.scalar] if alt_dge else [nc.sync]
                rows = TH // load_split
                for li in range(load_split):
                    r0 = li * rows
                    r1 = TH if li == load_split - 1 else r0 + rows
                    dges[li % len(dges)].dma_start(
                        xt[:, r0:r1, :], x_d[g0 : g0 + 128, r0:r1, :]
                    )

            for ht in range(NHT):
                h0 = ht * HT
                for r in rots:
                    k_pe, m_dve = split[r]
                    osb = opool.tile([128, HT, W], BF16, tag="osb")
                    ti_ = 0
                    if k_pe > 0:
                        for s in range(NSUB):
                            ps = pspool.tile([128, SUB, W], F32, tag="ps")
                            for k in range(k_pe):
                                ti, tj = TAPS[k]
                                nc.tensor.matmul(
                                    ps[:],
                                    diag[:, r * 9 + ti * 3 + tj, :],
                                    xt[:, h0 + s * SUB + ti : h0 + s * SUB + ti + SUB, tj : tj + W],
                                    start=(k == 0),
                                    stop=(k == k_pe - 1),
                                )
                            nc.scalar.activation(
                                osb[:, s * SUB : (s + 1) * SUB, :],
                                ps[:],
                                mybir.ActivationFunctionType.Copy,
                            )
                        ti_ = k_pe
                    else:
                        nc.scalar.activation(
                            osb[:],
                            xt[:, h0 + 1 : h0 + 1 + HT, 1 : 1 + W],
                            mybir.ActivationFunctionType.Copy,
                            scale=w_sb[:, r * 9 + 4 : r * 9 + 5],
                        )
                        ti_ = 1
                    for k in range(ti_, ti_ + m_dve):
                        ti, tj = TAPS[k]
                        nc.vector.scalar_tensor_tensor(
                            out=osb[:],
                            in0=xt[:, h0 + ti : h0 + ti + HT, tj : tj + W],
                            scalar=w_sb[:, r * 9 + ti * 3 + tj : r * 9 + ti * 3 + tj + 1],
                            in1=osb[:],
                            op0=mybir.AluOpType.mult,
                            op1=mybir.AluOpType.add,
                        )
                    if do_stores:
                        for si, (p0, n, b_loc, c0) in enumerate(CHUNK_SEGS[ch]):
                            row0 = b_loc * 4 * C + r * C + c0
                            dge = nc.scalar if (alt_dge and (r + si) % 2) else nc.sync
                            dge.dma_start(
                                o_d[row0 : row0 + n, h0 : h0 + HT, :],
                                osb[p0 : p0 + n, :, :],
                            )

    nc.compile()
    return nc


_NC = None


def _get_nc():
    global _NC
    if _NC is None:
        _NC = _build()
    return _NC


def _make_w36(weight):
    w36 = np.zeros((C, 36), dtype=np.float32)
    base = weight[:, 0]
    for r in range(4):
        wr = np.rot90(base, r, axes=(1, 2))
        w36[:, r * 9 : (r + 1) * 9] = wr.reshape(C, 9)
    return np.tile(w36, (BS, 1))


def make_in_maps(x, weight):
    bf = mybir.dt.np(BF16)
    w36 = _make_w36(weight)
    w36b = w36.astype(bf)
    xp = np.zeros((B, C, TH, TW), dtype=bf)
    xp[:, :, 1 : H + 1, 1 : W + 1] = x.astype(bf)
    return [
        {
            "x": np.ascontiguousarray(xp[BS * k : BS * (k + 1)].reshape(ROWS, TH, TW)),
            "w36": w36,
            "w36b": w36b,
        }
        for k in range(NCORES)
    ]


def kernel(x, weight):
    x = np.asarray(x, dtype=np.float32)
    weight = np.asarray(weight, dtype=np.float32)
    in_maps = make_in_maps(x, weight)
    nc = _get_nc()
    res = run_bass_kernel_spmd(nc, in_maps, list(range(NCORES))).results
    out = np.empty((B, 4 * C, H, W), dtype=np.float32)
    for k in range(NCORES):
        out[BS * k : BS * (k + 1)] = (
            res[k]["out"].astype(np.float32).reshape(BS, 4 * C, H, W)
        )
    return out
